# revision 12
# baseline (speedup 1.0000x reference)
"""Trainium2 Bass kernel for a GPT-style causal multi-head attention block.

Reference computation (per problem nn_Attention_45286135169078):
    qkv = x @ c_attn_w + c_attn_b              # [B,S,3D]
    q,k,v -> heads [B,H,S,hd], causal softmax(q k^T / sqrt(hd)) @ v
    a = merge_heads @ c_proj_w + c_proj_b      # [B,S,D]
    present = stack(k_heads, v_heads)          # [2,B,H,S,hd]
    returns (a, present)

Sharding across 8 NeuronCores: (batch b, head-group hg) with b in {0,1} and
hg in {0..3}; each core handles 4 heads of one batch (tensor-parallel over
heads x data-parallel over batch).  c_attn columns / c_proj rows are split by
head on the host; the c_proj partial outputs are summed on the host (the
"all-reduce after c_proj" of the hint, done at gather time).

Per-core device kernel (all matmuls on fp32 data run in float32r mode, the
exp->AV path runs in bf16):
  qk^T = (x Wqk)^T  [512, 2048]  - q rows 0..255, k rows 256..511, with the
                                   two heads of a "pair" stacked in one
                                   128-partition tile
  v    = x Wv       [2048, 256]  - natural layout, plus a ones column per
                                   head -> AV matmul also produces softmax
                                   denominators (M=65)
  scores^T blocks [128 j, 512 i] - lhsT = k^T slice (K=64), two heads packed
                                   into the PE array via row tile_position
  e = exp(scores/8)  on ScalarE, PSUM->SBUF, bf16, 2 blocks per instruction
  causal masking     0/1 bf16 mask multiply on diagonal blocks only
  AV: lhsT = [v | 1] [128, 65], rhs = e block -> accumulate [65, 512] in PSUM
  softmax division:  reciprocal of row 64, broadcast via K=1 matmul with a
                     ones vector, multiply on VectorE
  proj partial:      lhsT = a^T pair tile [128, 128], rhs = c_proj slice
"""

import os
import sys

import numpy as np

if "/opt/trn_rl_repo" not in sys.path:
    sys.path.insert(0, "/opt/trn_rl_repo")

import ml_dtypes

import concourse.bass as bass
import concourse.mybir as mybir
import concourse.tile as tile
from concourse import bacc
from concourse.bass_utils import run_bass_kernel_spmd

F32 = mybir.dt.float32
F32R = mybir.dt.float32r
BF16 = mybir.dt.bfloat16

B, S, D, H, HD = 2, 2048, 1024, 16, 64
N_CORES = 8
HPC = 4                       # heads per core
QKR = 2 * HPC * HD            # qk^T rows per core (q:256 + k:256) = 512
VC = HPC * HD                 # v columns per core = 256
NKC = D // 128                # contraction chunks over embedding = 8
NJC = S // 128                # key/seq chunks of 128 = 16
NIC = S // 512                # query chunks of 512 = 4
SCALE = 0.125                 # 1/sqrt(hd)

_CACHE: dict = {}


def _build_masks_bf16() -> np.ndarray:
    """masks[m][p, f] = 1.0 if (f >= p + 128*m) else 0 - the causal mask for a
    scores^T block whose key chunk is the (4*ic + m)-th within query chunk ic."""
    p = np.arange(128)[:, None]
    f = np.arange(512)[None, :]
    out = np.zeros((4, 128, 512), np.float32)
    for m in range(4):
        out[m] = (f >= p + 128 * m).astype(np.float32)
    return out.astype(ml_dtypes.bfloat16)


def _emit(tc: tile.TileContext, ctx, tensors):
    nc = tc.nc
    xT_d, wqk_d, wv_d, bqk_d, bv_d, wp_d, ones_d, masks_d, out_d, kT_d, v_d = tensors

    def r(ap):
        return ap

    persist = ctx.enter_context(tc.tile_pool(name="persist", bufs=1))
    epool = ctx.enter_context(tc.tile_pool(name="epool", bufs=6))
    small = ctx.enter_context(tc.tile_pool(name="small", bufs=3))
    outp = ctx.enter_context(tc.tile_pool(name="outp", bufs=2))
    ps_mm = ctx.enter_context(tc.tile_pool(name="ps_mm", bufs=2, space="PSUM"))
    ps_sc = ctx.enter_context(tc.tile_pool(name="ps_sc", bufs=2, space="PSUM"))
    ps_avA = ctx.enter_context(tc.tile_pool(name="ps_avA", bufs=1, space="PSUM"))
    ps_avB = ctx.enter_context(tc.tile_pool(name="ps_avB", bufs=1, space="PSUM"))

    # ---- persistent SBUF tiles -------------------------------------------
    wqk_sb = persist.tile([128, NKC, QKR], F32R, tag="wqk", name="wqk_sb")
    wv_sb = persist.tile([128, NKC, VC], F32R, tag="wv", name="wv_sb")
    bqk_sb = persist.tile([128, QKR // 128], F32, tag="bqk", name="bqk_sb")
    bv_sb = persist.tile([1, VC], F32R, tag="bv", name="bv_sb")
    wp_sb = persist.tile([128, 2, D], F32R, tag="wp", name="wp_sb")
    masks_sb = persist.tile([128, 4, 512], BF16, tag="masks", name="masks_sb")
    ones_sb = persist.tile([128, 128], F32R, tag="ones", name="ones_sb")
    xT_sb = persist.tile([128, NKC, S], F32R, tag="xT", name="xT_sb")
    # qk^T tiles: [0]=q heads(0,1), [1]=q heads(2,3), [2]=k heads(0,1), [3]=k(2,3)
    qkT_sb = [
        persist.tile([128, S], F32R, tag=f"qkT{m}", name=f"qkT{m}") for m in range(4)
    ]
    # v with ones column, bf16: [128p(seq within chunk), jc, head, 65]
    v65_sb = persist.tile([128, NJC, HPC, 65], BF16, tag="v65", name="v65_sb")
    # a^T per head pair: rows 0-63 head 2*pair dims, 64-127 head 2*pair+1
    aT_sb = [
        persist.tile([128, S], F32R, tag=f"aT{p}", name=f"aT{p}") for p in range(2)
    ]

    # ---- input DMAs -------------------------------------------------------
    # order matters for the head of the schedule: wqk + the n=0 quarter of
    # xT land first so the first qk^T pass can start ~7us in.
    wqk_r = wqk_d.rearrange("(ko p) m -> p ko m", p=128)
    for m in (0, 2):
        nc.sync.dma_start(wqk_sb[:, :, m * 128:(m + 1) * 128],
                          wqk_r[:, :, m * 128:(m + 1) * 128])
    for n in range(4):
        for k in range(NKC):
            eng = nc.sync if k % 2 == 0 else nc.scalar
            eng.dma_start(
                xT_sb[:, k, n * 512:(n + 1) * 512],
                xT_d[k * 128:(k + 1) * 128, n * 512:(n + 1) * 512],
            )
        if n == 0:
            for m in (1, 3):
                nc.sync.dma_start(wqk_sb[:, :, m * 128:(m + 1) * 128],
                                  wqk_r[:, :, m * 128:(m + 1) * 128])
            nc.sync.dma_start(bqk_sb[:], bqk_d.rearrange("(m p) -> p m", p=128))
            nc.sync.dma_start(wv_sb[:], wv_d.rearrange("(ko p) m -> p ko m", p=128))
            nc.sync.dma_start(ones_sb[:], ones_d[:])
            nc.sync.dma_start(bv_sb[:], bv_d[None, :])
            nc.sync.dma_start(masks_sb[:], masks_d.rearrange("m p f -> p m f"))
        if n == 1:
            nc.sync.dma_start(wp_sb[:], wp_d.rearrange("(ko p) n -> p ko n", p=128))
    nc.vector.memset(v65_sb[:, :, :, 64:65], 1.0)

    def emit_qkT(m, n):
        ps = ps_mm.tile([128, 512], F32, tag="mm", name=f"qk_ps_{m}_{n}")
        for k in range(NKC):
            nc.tensor.matmul(
                ps[:],
                wqk_sb[:, k, m * 128:(m + 1) * 128],
                xT_sb[:, k, n * 512:(n + 1) * 512],
                start=(k == 0),
                stop=(k == NKC - 1),
            )
        nc.vector.tensor_scalar(
            qkT_sb[m][:, n * 512:(n + 1) * 512],
            ps[:],
            bqk_sb[:, m:m + 1],
            None,
            mybir.AluOpType.add,
        )

    def emit_v(sc):
        ps = ps_mm.tile([128, 512], F32, tag="mm", name=f"v_ps_{sc}")
        psv = ps[:, 0:VC]
        for k in range(NKC):
            nc.tensor.matmul(
                psv,
                xT_sb[:, k, sc * 128:(sc + 1) * 128],
                wv_sb[:, k, :],
                start=(k == 0),
                stop=False,
            )
        # bias via K=1 rank-1 update: ones[128] x bv[256]
        nc.tensor.matmul(
            psv, ones_sb[0:1, 0:128], bv_sb[0:1, :], start=False, stop=True
        )
        vout = outp.tile([128, VC], F32, tag="vout", name=f"vout_{sc}")
        nc.vector.tensor_copy(vout[:], psv)
        nc.gpsimd.dma_start(v_d[sc * 128:(sc + 1) * 128, :], vout[:])
        nc.vector.tensor_copy(
            v65_sb[:, sc, :, 0:64], psv.rearrange("p (h d) -> p h d", h=HPC)
        )

    def emit_attention(pair, ic):
        qT = qkT_sb[pair]
        kT = qkT_sb[2 + pair]
        aT = aT_sb[pair]
        njc = 4 * ic + 4
        avA = ps_avA.tile([65, 512], F32, tag="avA", name=f"avA_{pair}_{ic}")
        avB = ps_avB.tile([65, 512], F32, tag="avB", name=f"avB_{pair}_{ic}")
        for jc in range(njc):
            # one 2-bank scores tile per key chunk: [A, B]; bufs=2 pipelines
            # the next chunk's matmuls against this chunk's exp
            sc2 = ps_sc.tile([128, 2, 512], F32, tag="sc", name=f"sc_{pair}_{ic}_{jc}")
            nc.tensor.matmul(
                sc2[:, 0, :],
                kT[0:64, jc * 128:(jc + 1) * 128],
                qT[0:64, ic * 512:(ic + 1) * 512],
            )
            nc.tensor.matmul(
                sc2[:, 1, :],
                kT[64:128, jc * 128:(jc + 1) * 128],
                qT[64:128, ic * 512:(ic + 1) * 512],
            )
            e2 = epool.tile([128, 2, 512], BF16, tag="e", name=f"e_{pair}_{ic}_{jc}")
            nc.scalar.activation(
                e2[:], sc2[:], mybir.ActivationFunctionType.Exp, scale=SCALE
            )
            mloc = jc - 4 * ic
            if mloc >= 0:  # diagonal block: zero out future positions
                nc.vector.tensor_tensor(
                    e2[:, 0, :], e2[:, 0, :], masks_sb[:, mloc, :],
                    mybir.AluOpType.mult,
                )
                nc.vector.tensor_tensor(
                    e2[:, 1, :], e2[:, 1, :], masks_sb[:, mloc, :],
                    mybir.AluOpType.mult,
                )
            nc.tensor.matmul(
                avA[:],
                v65_sb[:, jc, 2 * pair, :],
                e2[:, 0, :],
                start=(jc == 0),
                stop=(jc == njc - 1),
            )
            nc.tensor.matmul(
                avB[:],
                v65_sb[:, jc, 2 * pair + 1, :],
                e2[:, 1, :],
                start=(jc == 0),
                stop=(jc == njc - 1),
            )
        # epilogue: divide by the softmax denominator (row 64)
        ics = slice(ic * 512, (ic + 1) * 512)
        for hb, av in ((0, avA), (1, avB)):
            rr = small.tile([65, 512], F32R, tag="recip", name=f"r_{pair}_{ic}_{hb}")
            with nc.allow_low_precision(reason="fp32r rhs of broadcast matmul"):
                nc.vector.reciprocal(rr[64:65, :], av[64:65, :])
            bc_ps = ps_mm.tile([128, 512], F32, tag="mm", name=f"bc_ps_{pair}_{ic}_{hb}")
            nc.tensor.matmul(bc_ps[:], ones_sb[64:65, 0:128], rr[64:65, :])
            bc = small.tile([128, 512], F32, tag="bcast", name=f"bc_{pair}_{ic}_{hb}")
            nc.vector.tensor_copy(bc[:], bc_ps[:])
            if hb == 0:
                nc.vector.tensor_tensor(
                    aT[0:64, ics], av[0:64, :], bc[0:64, :], mybir.AluOpType.mult
                )
            else:
                tmp = small.tile([64, 512], F32R, tag="tmpB", name=f"tmpB_{pair}_{ic}")
                nc.vector.tensor_tensor(
                    tmp[0:64, :], av[0:64, :], bc[0:64, :], mybir.AluOpType.mult
                )
                # shift to partitions 64-127 of the pair tile (DMA crossbar)
                nc.scalar.dma_start(aT[64:128, ics], tmp[0:64, :])

    def emit_proj(i):
        osb = outp.tile([128, D], F32, tag="osb", name=f"osb_{i}")
        for n in range(2):
            ps = ps_mm.tile([128, 512], F32, tag="mm", name=f"o_ps_{i}_{n}")
            for kc in range(2):
                nc.tensor.matmul(
                    ps[:],
                    aT_sb[kc][:, i * 128:(i + 1) * 128],
                    wp_sb[:, kc, n * 512:(n + 1) * 512],
                    start=(kc == 0),
                    stop=(kc == 1),
                )
            nc.vector.tensor_copy(osb[:, n * 512:(n + 1) * 512], ps[:])
        nc.gpsimd.dma_start(out_d[i * 128:(i + 1) * 128, :], osb[:])

    # ---- main schedule: ic-outer; the next chunk's qk^T/v are emitted
    # before this chunk's proj so the PE always has projection work queued
    # while the attention epilogues drain.
    for m in (0, 2, 1, 3):
        emit_qkT(m, 0)
    for sc in range(4):
        emit_v(sc)
    for ic in range(NIC):
        for pair in range(2):
            emit_attention(pair, ic)
        if ic + 1 < NIC:
            for m in (0, 2, 1, 3):
                emit_qkT(m, ic + 1)
            for sc in range(4 * ic + 4, 4 * ic + 8):
                emit_v(sc)
        for i in range(4 * ic, 4 * ic + 4):
            emit_proj(i)

    # present: k^T rows are qkT tiles 2 and 3
    nc.gpsimd.dma_start(kT_d[0:128, :], qkT_sb[2][:])
    nc.gpsimd.dma_start(kT_d[128:256, :], qkT_sb[3][:])


def build_nc():
    if "nc" in _CACHE:
        return _CACHE["nc"]
    from contextlib import ExitStack

    nc = bacc.Bacc(None, target_bir_lowering=False)
    xT_d = nc.dram_tensor("xT", [D, S], F32R, kind="ExternalInput")
    wqk_d = nc.dram_tensor("wqk", [D, QKR], F32R, kind="ExternalInput")
    wv_d = nc.dram_tensor("wv", [D, VC], F32R, kind="ExternalInput")
    bqk_d = nc.dram_tensor("bqk", [QKR], F32, kind="ExternalInput")
    bv_d = nc.dram_tensor("bv", [VC], F32R, kind="ExternalInput")
    wp_d = nc.dram_tensor("wp", [VC, D], F32R, kind="ExternalInput")
    ones_d = nc.dram_tensor("ones", [128, 128], F32R, kind="ExternalInput")
    masks_d = nc.inline_tensor(_build_masks_bf16(), name="masks")
    out_d = nc.dram_tensor("out_p", [S, D], F32, kind="ExternalOutput")
    kT_d = nc.dram_tensor("kT_out", [VC, S], F32R, kind="ExternalOutput")
    v_d = nc.dram_tensor("v_out", [S, VC], F32, kind="ExternalOutput")

    tensors = (xT_d[:], wqk_d[:], wv_d[:], bqk_d[:], bv_d[:], wp_d[:],
               ones_d[:], masks_d[:], out_d[:], kT_d[:], v_d[:])
    with tile.TileContext(nc) as tc:
        with ExitStack() as ctx:
            _emit(tc, ctx, tensors)
    nc.compile()
    _CACHE["nc"] = nc
    return nc


def make_in_maps(x, c_attn_w, c_attn_b, c_proj_w):
    x = np.asarray(x, np.float32)
    c_attn_w = np.asarray(c_attn_w, np.float32)
    c_attn_b = np.asarray(c_attn_b, np.float32)
    c_proj_w = np.asarray(c_proj_w, np.float32)
    in_maps = []
    for c in range(N_CORES):
        b, hg = divmod(c, 4)
        qs = slice(VC * hg, VC * (hg + 1))
        ks = slice(D + VC * hg, D + VC * (hg + 1))
        vs = slice(2 * D + VC * hg, 2 * D + VC * (hg + 1))
        in_maps.append({
            "xT": np.ascontiguousarray(x[b].T),
            "wqk": np.ascontiguousarray(
                np.concatenate([c_attn_w[:, qs], c_attn_w[:, ks]], axis=1)),
            "wv": np.ascontiguousarray(c_attn_w[:, vs]),
            "bqk": np.ascontiguousarray(
                np.concatenate([c_attn_b[qs], c_attn_b[ks]])),
            "bv": np.ascontiguousarray(c_attn_b[vs]),
            "wp": np.ascontiguousarray(c_proj_w[VC * hg:VC * (hg + 1), :]),
            "ones": np.ones((128, 128), np.float32),
        })
    return in_maps


def gather(results, c_proj_b):
    c_proj_b = np.asarray(c_proj_b, np.float32)
    a = np.zeros((B, S, D), np.float32)
    present = np.empty((2, B, H, S, HD), np.float32)
    for c in range(N_CORES):
        b, hg = divmod(c, 4)
        rs = results[c]
        a[b] += rs["out_p"]
        hsl = slice(HPC * hg, HPC * (hg + 1))
        present[0, b, hsl] = (
            rs["kT_out"].reshape(HPC, HD, S).transpose(0, 2, 1))
        present[1, b, hsl] = (
            rs["v_out"].reshape(S, HPC, HD).transpose(1, 0, 2))
    a += c_proj_b
    return a, present


def run(in_maps, trace=False, **kw):
    nc = build_nc()
    if not trace:
        # this container has no NTFF hook (antenv.axon_hooks absent); a
        # BASS_TRACE env var would crash the axon trace path, so pin it off
        os.environ.setdefault("BASS_NEVER_TRACE", "1")
    return run_bass_kernel_spmd(nc, in_maps, list(range(N_CORES)), trace=trace, **kw)


def kernel(x, c_attn_w, c_attn_b, c_proj_w, c_proj_b):
    in_maps = make_in_maps(x, c_attn_w, c_attn_b, c_proj_w)
    res = run(in_maps)
    return gather(res.results, c_proj_b)


# revision 17
# speedup vs baseline: 1.0028x; 1.0028x over previous
"""Trainium2 Bass kernel for a GPT-style causal multi-head attention block.

Reference computation (per problem nn_Attention_45286135169078):
    qkv = x @ c_attn_w + c_attn_b              # [B,S,3D]
    q,k,v -> heads [B,H,S,hd], causal softmax(q k^T / sqrt(hd)) @ v
    a = merge_heads @ c_proj_w + c_proj_b      # [B,S,D]
    present = stack(k_heads, v_heads)          # [2,B,H,S,hd]
    returns (a, present)

Sharding across 8 NeuronCores: (batch b, head-group hg) with b in {0,1} and
hg in {0..3}; each core handles 4 heads of one batch (tensor-parallel over
heads x data-parallel over batch).  c_attn columns / c_proj rows are split by
head on the host; the c_proj partial outputs are summed on the host (the
"all-reduce after c_proj" of the hint, done at gather time).

Per-core device kernel (all matmuls on fp32 data run in float32r mode, the
exp->AV path runs in bf16):
  qk^T = (x Wqk)^T  [512, 2048]  - q rows 0..255, k rows 256..511, with the
                                   two heads of a "pair" stacked in one
                                   128-partition tile
  v    = x Wv       [2048, 256]  - natural layout, plus a ones column per
                                   head -> AV matmul also produces softmax
                                   denominators (M=65)
  scores^T blocks [128 j, 512 i] - lhsT = k^T slice (K=64), two heads packed
                                   into the PE array via row tile_position
  e = exp(scores/8)  on ScalarE, PSUM->SBUF, bf16, 2 blocks per instruction
  causal masking     0/1 bf16 mask multiply on diagonal blocks only
  AV: lhsT = [v | 1] [128, 65], rhs = e block -> accumulate [65, 512] in PSUM
  softmax division:  reciprocal of row 64, broadcast via K=1 matmul with a
                     ones vector, multiply on VectorE
  proj partial:      lhsT = a^T pair tile [128, 128], rhs = c_proj slice
"""

import os
import sys

import numpy as np

if "/opt/trn_rl_repo" not in sys.path:
    sys.path.insert(0, "/opt/trn_rl_repo")

import ml_dtypes

import concourse.bass as bass
import concourse.mybir as mybir
import concourse.tile as tile
from concourse import bacc
from concourse.bass_utils import run_bass_kernel_spmd

F32 = mybir.dt.float32
F32R = mybir.dt.float32r
BF16 = mybir.dt.bfloat16

B, S, D, H, HD = 2, 2048, 1024, 16, 64
N_CORES = 8
HPC = 4                       # heads per core
QKR = 2 * HPC * HD            # qk^T rows per core (q:256 + k:256) = 512
VC = HPC * HD                 # v columns per core = 256
NKC = D // 128                # contraction chunks over embedding = 8
NJC = S // 128                # key/seq chunks of 128 = 16
NIC = S // 512                # query chunks of 512 = 4
SCALE = 0.125                 # 1/sqrt(hd)

_CACHE: dict = {}


def _build_masks_bf16() -> np.ndarray:
    """masks[m][p, f] = 1.0 if (f >= p + 128*m) else 0 - the causal mask for a
    scores^T block whose key chunk is the (4*ic + m)-th within query chunk ic."""
    p = np.arange(128)[:, None]
    f = np.arange(512)[None, :]
    out = np.zeros((4, 128, 512), np.float32)
    for m in range(4):
        out[m] = (f >= p + 128 * m).astype(np.float32)
    return out.astype(ml_dtypes.bfloat16)


def _emit(tc: tile.TileContext, ctx, tensors):
    nc = tc.nc
    xT_d, wqk_d, wv_d, bqk_d, bv_d, wp_d, ones_d, masks_d, out_d, kT_d, v_d = tensors

    def r(ap):
        return ap

    persist = ctx.enter_context(tc.tile_pool(name="persist", bufs=1))
    epool = ctx.enter_context(tc.tile_pool(name="epool", bufs=6))
    small = ctx.enter_context(tc.tile_pool(name="small", bufs=3))
    outp = ctx.enter_context(tc.tile_pool(name="outp", bufs=2))
    ps_mm = ctx.enter_context(tc.tile_pool(name="ps_mm", bufs=2, space="PSUM"))
    ps_sc = ctx.enter_context(tc.tile_pool(name="ps_sc", bufs=2, space="PSUM"))
    ps_avA = ctx.enter_context(tc.tile_pool(name="ps_avA", bufs=1, space="PSUM"))
    ps_avB = ctx.enter_context(tc.tile_pool(name="ps_avB", bufs=1, space="PSUM"))

    # ---- persistent SBUF tiles -------------------------------------------
    wqk_sb = persist.tile([128, NKC, QKR], F32R, tag="wqk", name="wqk_sb")
    wv_sb = persist.tile([128, NKC, VC], F32R, tag="wv", name="wv_sb")
    bqk_sb = persist.tile([128, QKR // 128], F32, tag="bqk", name="bqk_sb")
    bv_sb = persist.tile([1, VC], F32R, tag="bv", name="bv_sb")
    wp_sb = persist.tile([128, 2, D], F32R, tag="wp", name="wp_sb")
    masks_sb = persist.tile([128, 4, 512], BF16, tag="masks", name="masks_sb")
    ones_sb = persist.tile([128, 128], F32R, tag="ones", name="ones_sb")
    xT_sb = persist.tile([128, NKC, S], F32R, tag="xT", name="xT_sb")
    # qk^T tiles: [0]=q heads(0,1), [1]=q heads(2,3), [2]=k heads(0,1), [3]=k(2,3)
    qkT_sb = [
        persist.tile([128, S], F32R, tag=f"qkT{m}", name=f"qkT{m}") for m in range(4)
    ]
    # v with ones column, bf16: [128p(seq within chunk), jc, head, 65]
    v65_sb = persist.tile([128, NJC, HPC, 65], BF16, tag="v65", name="v65_sb")
    # a^T per head pair: rows 0-63 head 2*pair dims, 64-127 head 2*pair+1
    aT_sb = [
        persist.tile([128, S], F32R, tag=f"aT{p}", name=f"aT{p}") for p in range(2)
    ]

    # ---- input DMAs -------------------------------------------------------
    # order matters for the head of the schedule: wqk + the n=0 quarter of
    # xT land first so the first qk^T pass can start ~7us in.
    wqk_r = wqk_d.rearrange("(ko p) m -> p ko m", p=128)
    nc.scalar.dma_start(wqk_sb[:, :, 0:128], wqk_r[:, :, 0:128])
    for n in range(4):
        for k in range(NKC):
            eng = nc.sync if k % 2 == 0 else nc.scalar
            eng.dma_start(
                xT_sb[:, k, n * 512:(n + 1) * 512],
                xT_d[k * 128:(k + 1) * 128, n * 512:(n + 1) * 512],
            )
        if n == 0:
            nc.sync.dma_start(wqk_sb[:, :, 256:384], wqk_r[:, :, 256:384])
            for m in (1, 3):
                nc.scalar.dma_start(wqk_sb[:, :, m * 128:(m + 1) * 128],
                                    wqk_r[:, :, m * 128:(m + 1) * 128])
            nc.sync.dma_start(bqk_sb[:], bqk_d.rearrange("(m p) -> p m", p=128))
            nc.sync.dma_start(wv_sb[:], wv_d.rearrange("(ko p) m -> p ko m", p=128))
            nc.sync.dma_start(ones_sb[:], ones_d[:])
            nc.sync.dma_start(bv_sb[:], bv_d[None, :])
            nc.sync.dma_start(masks_sb[:], masks_d.rearrange("m p f -> p m f"))
        if n == 1:
            nc.sync.dma_start(wp_sb[:], wp_d.rearrange("(ko p) n -> p ko n", p=128))
    nc.vector.memset(v65_sb[:, :, :, 64:65], 1.0)
    # warm the ScalarE exp table during the DMA phase (~2.7us table load)
    warm = small.tile([1, 4], F32, tag="warm", name="warm_sb")
    nc.vector.memset(warm[:], 0.0)
    nc.scalar.activation(warm[:], warm[:], mybir.ActivationFunctionType.Exp)

    def emit_qkT(m, n):
        ps = ps_mm.tile([128, 512], F32, tag="mm", name=f"qk_ps_{m}_{n}")
        for k in range(NKC):
            nc.tensor.matmul(
                ps[:],
                wqk_sb[:, k, m * 128:(m + 1) * 128],
                xT_sb[:, k, n * 512:(n + 1) * 512],
                start=(k == 0),
                stop=(k == NKC - 1),
            )
        nc.vector.tensor_scalar(
            qkT_sb[m][:, n * 512:(n + 1) * 512],
            ps[:],
            bqk_sb[:, m:m + 1],
            None,
            mybir.AluOpType.add,
        )

    def emit_v(sc):
        ps = ps_mm.tile([128, 512], F32, tag="mm", name=f"v_ps_{sc}")
        psv = ps[:, 0:VC]
        for k in range(NKC):
            nc.tensor.matmul(
                psv,
                xT_sb[:, k, sc * 128:(sc + 1) * 128],
                wv_sb[:, k, :],
                start=(k == 0),
                stop=False,
            )
        # bias via K=1 rank-1 update: ones[128] x bv[256]
        nc.tensor.matmul(
            psv, ones_sb[0:1, 0:128], bv_sb[0:1, :], start=False, stop=True
        )
        vout = outp.tile([128, VC], F32, tag="vout", name=f"vout_{sc}")
        nc.vector.tensor_copy(vout[:], psv)
        nc.gpsimd.dma_start(v_d[sc * 128:(sc + 1) * 128, :], vout[:])
        nc.vector.tensor_copy(
            v65_sb[:, sc, :, 0:64], psv.rearrange("p (h d) -> p h d", h=HPC)
        )

    def emit_attention(pair, ic):
        qT = qkT_sb[pair]
        kT = qkT_sb[2 + pair]
        aT = aT_sb[pair]
        njc = 4 * ic + 4
        avA = ps_avA.tile([65, 512], F32, tag="avA", name=f"avA_{pair}_{ic}")
        avB = ps_avB.tile([65, 512], F32, tag="avB", name=f"avB_{pair}_{ic}")
        for jc in range(njc):
            # one 2-bank scores tile per key chunk: [A, B]; bufs=2 pipelines
            # the next chunk's matmuls against this chunk's exp
            sc2 = ps_sc.tile([128, 2, 512], F32, tag="sc", name=f"sc_{pair}_{ic}_{jc}")
            nc.tensor.matmul(
                sc2[:, 0, :],
                kT[0:64, jc * 128:(jc + 1) * 128],
                qT[0:64, ic * 512:(ic + 1) * 512],
            )
            nc.tensor.matmul(
                sc2[:, 1, :],
                kT[64:128, jc * 128:(jc + 1) * 128],
                qT[64:128, ic * 512:(ic + 1) * 512],
            )
            e2 = epool.tile([128, 2, 512], BF16, tag="e", name=f"e_{pair}_{ic}_{jc}")
            nc.scalar.activation(
                e2[:], sc2[:], mybir.ActivationFunctionType.Exp, scale=SCALE
            )
            mloc = jc - 4 * ic
            if mloc >= 0:  # diagonal block: zero out future positions
                nc.vector.tensor_tensor(
                    e2[:, :, :], e2[:, :, :],
                    masks_sb[:, mloc:mloc + 1, :].to_broadcast([128, 2, 512]),
                    mybir.AluOpType.mult,
                )
            nc.tensor.matmul(
                avA[:],
                v65_sb[:, jc, 2 * pair, :],
                e2[:, 0, :],
                start=(jc == 0),
                stop=(jc == njc - 1),
            )
            nc.tensor.matmul(
                avB[:],
                v65_sb[:, jc, 2 * pair + 1, :],
                e2[:, 1, :],
                start=(jc == 0),
                stop=(jc == njc - 1),
            )
        # epilogue: divide by the softmax denominator (row 64)
        ics = slice(ic * 512, (ic + 1) * 512)
        for hb, av in ((0, avA), (1, avB)):
            rr = small.tile([65, 512], F32R, tag="recip", name=f"r_{pair}_{ic}_{hb}")
            with nc.allow_low_precision(reason="fp32r rhs of broadcast matmul"):
                nc.vector.reciprocal(rr[64:65, :], av[64:65, :])
            bc_ps = ps_mm.tile([128, 512], F32, tag="mm", name=f"bc_ps_{pair}_{ic}_{hb}")
            nc.tensor.matmul(bc_ps[:], ones_sb[64:65, 0:128], rr[64:65, :])
            bc = small.tile([128, 512], F32, tag="bcast", name=f"bc_{pair}_{ic}_{hb}")
            nc.vector.tensor_copy(bc[:], bc_ps[:])
            if hb == 0:
                nc.vector.tensor_tensor(
                    aT[0:64, ics], av[0:64, :], bc[0:64, :], mybir.AluOpType.mult
                )
            else:
                tmp = small.tile([64, 512], F32R, tag="tmpB", name=f"tmpB_{pair}_{ic}")
                nc.vector.tensor_tensor(
                    tmp[0:64, :], av[0:64, :], bc[0:64, :], mybir.AluOpType.mult
                )
                # shift to partitions 64-127 of the pair tile (DMA crossbar)
                nc.scalar.dma_start(aT[64:128, ics], tmp[0:64, :])

    def emit_proj(i):
        osb = outp.tile([128, D], F32, tag="osb", name=f"osb_{i}")
        for n in range(2):
            ps = ps_mm.tile([128, 512], F32, tag="mm", name=f"o_ps_{i}_{n}")
            for kc in range(2):
                nc.tensor.matmul(
                    ps[:],
                    aT_sb[kc][:, i * 128:(i + 1) * 128],
                    wp_sb[:, kc, n * 512:(n + 1) * 512],
                    start=(kc == 0),
                    stop=(kc == 1),
                )
            nc.vector.tensor_copy(osb[:, n * 512:(n + 1) * 512], ps[:])
        nc.gpsimd.dma_start(out_d[i * 128:(i + 1) * 128, :], osb[:])

    # ---- main schedule: ic-outer; the next chunk's qk^T/v are emitted
    # before this chunk's proj so the PE always has projection work queued
    # while the attention epilogues drain.
    for m in (0, 2, 1, 3):
        emit_qkT(m, 0)
    for sc in range(4):
        emit_v(sc)
    for ic in range(NIC):
        for pair in range(2):
            emit_attention(pair, ic)
        if ic + 1 < NIC:
            for m in (0, 2, 1, 3):
                emit_qkT(m, ic + 1)
            for sc in range(4 * ic + 4, 4 * ic + 8):
                emit_v(sc)
        if ic + 1 == NIC - 1:
            # present: k^T rows are qkT tiles 2 and 3 - emit as soon as the
            # last qk^T passes exist so the DMA drains during attention
            nc.gpsimd.dma_start(kT_d[0:128, :], qkT_sb[2][:])
            nc.gpsimd.dma_start(kT_d[128:256, :], qkT_sb[3][:])
        for i in range(4 * ic, 4 * ic + 4):
            emit_proj(i)


def build_nc():
    if "nc" in _CACHE:
        return _CACHE["nc"]
    from contextlib import ExitStack

    nc = bacc.Bacc(None, target_bir_lowering=False)
    xT_d = nc.dram_tensor("xT", [D, S], F32R, kind="ExternalInput")
    wqk_d = nc.dram_tensor("wqk", [D, QKR], F32R, kind="ExternalInput")
    wv_d = nc.dram_tensor("wv", [D, VC], F32R, kind="ExternalInput")
    bqk_d = nc.dram_tensor("bqk", [QKR], F32, kind="ExternalInput")
    bv_d = nc.dram_tensor("bv", [VC], F32R, kind="ExternalInput")
    wp_d = nc.dram_tensor("wp", [VC, D], F32R, kind="ExternalInput")
    ones_d = nc.dram_tensor("ones", [128, 128], F32R, kind="ExternalInput")
    masks_d = nc.inline_tensor(_build_masks_bf16(), name="masks")
    out_d = nc.dram_tensor("out_p", [S, D], F32, kind="ExternalOutput")
    kT_d = nc.dram_tensor("kT_out", [VC, S], F32R, kind="ExternalOutput")
    v_d = nc.dram_tensor("v_out", [S, VC], F32, kind="ExternalOutput")

    tensors = (xT_d[:], wqk_d[:], wv_d[:], bqk_d[:], bv_d[:], wp_d[:],
               ones_d[:], masks_d[:], out_d[:], kT_d[:], v_d[:])
    with tile.TileContext(nc) as tc:
        with ExitStack() as ctx:
            _emit(tc, ctx, tensors)
    nc.compile()
    _CACHE["nc"] = nc
    return nc


def make_in_maps(x, c_attn_w, c_attn_b, c_proj_w):
    x = np.asarray(x, np.float32)
    c_attn_w = np.asarray(c_attn_w, np.float32)
    c_attn_b = np.asarray(c_attn_b, np.float32)
    c_proj_w = np.asarray(c_proj_w, np.float32)
    in_maps = []
    for c in range(N_CORES):
        b, hg = divmod(c, 4)
        qs = slice(VC * hg, VC * (hg + 1))
        ks = slice(D + VC * hg, D + VC * (hg + 1))
        vs = slice(2 * D + VC * hg, 2 * D + VC * (hg + 1))
        in_maps.append({
            "xT": np.ascontiguousarray(x[b].T),
            "wqk": np.ascontiguousarray(
                np.concatenate([c_attn_w[:, qs], c_attn_w[:, ks]], axis=1)),
            "wv": np.ascontiguousarray(c_attn_w[:, vs]),
            "bqk": np.ascontiguousarray(
                np.concatenate([c_attn_b[qs], c_attn_b[ks]])),
            "bv": np.ascontiguousarray(c_attn_b[vs]),
            "wp": np.ascontiguousarray(c_proj_w[VC * hg:VC * (hg + 1), :]),
            "ones": np.ones((128, 128), np.float32),
        })
    return in_maps


def gather(results, c_proj_b):
    c_proj_b = np.asarray(c_proj_b, np.float32)
    a = np.zeros((B, S, D), np.float32)
    present = np.empty((2, B, H, S, HD), np.float32)
    for c in range(N_CORES):
        b, hg = divmod(c, 4)
        rs = results[c]
        a[b] += rs["out_p"]
        hsl = slice(HPC * hg, HPC * (hg + 1))
        present[0, b, hsl] = (
            rs["kT_out"].reshape(HPC, HD, S).transpose(0, 2, 1))
        present[1, b, hsl] = (
            rs["v_out"].reshape(S, HPC, HD).transpose(1, 0, 2))
    a += c_proj_b
    return a, present


def run(in_maps, trace=False, **kw):
    nc = build_nc()
    if not trace:
        # this container has no NTFF hook (antenv.axon_hooks absent); a
        # BASS_TRACE env var would crash the axon trace path, so pin it off
        os.environ.setdefault("BASS_NEVER_TRACE", "1")
    return run_bass_kernel_spmd(nc, in_maps, list(range(N_CORES)), trace=trace, **kw)


def kernel(x, c_attn_w, c_attn_b, c_proj_w, c_proj_b):
    in_maps = make_in_maps(x, c_attn_w, c_attn_b, c_proj_w)
    res = run(in_maps)
    return gather(res.results, c_proj_b)


# revision 19
# speedup vs baseline: 1.0209x; 1.0181x over previous
"""Trainium2 Bass kernel for a GPT-style causal multi-head attention block.

Reference computation (per problem nn_Attention_45286135169078):
    qkv = x @ c_attn_w + c_attn_b              # [B,S,3D]
    q,k,v -> heads [B,H,S,hd], causal softmax(q k^T / sqrt(hd)) @ v
    a = merge_heads @ c_proj_w + c_proj_b      # [B,S,D]
    present = stack(k_heads, v_heads)          # [2,B,H,S,hd]
    returns (a, present)

Sharding across 8 NeuronCores: (batch b, head-group hg) with b in {0,1} and
hg in {0..3}; each core handles 4 heads of one batch (tensor-parallel over
heads x data-parallel over batch).  c_attn columns / c_proj rows are split by
head on the host; the c_proj partial outputs are summed on the host (the
"all-reduce after c_proj" of the hint, done at gather time).

Per-core device kernel (all matmuls on fp32 data run in float32r mode, the
exp->AV path runs in bf16):
  qk^T = (x Wqk)^T  [512, 2048]  - q rows 0..255, k rows 256..511, with the
                                   two heads of a "pair" stacked in one
                                   128-partition tile
  v    = x Wv       [2048, 256]  - natural layout, plus a ones column per
                                   head -> AV matmul also produces softmax
                                   denominators (M=65)
  scores^T blocks [128 j, 512 i] - lhsT = k^T slice (K=64), two heads packed
                                   into the PE array via row tile_position
  e = exp(scores/8)  on ScalarE, PSUM->SBUF, bf16, 2 blocks per instruction
  causal masking     0/1 bf16 mask multiply on diagonal blocks only
  AV: lhsT = [v | 1] [128, 65], rhs = e block -> accumulate [65, 512] in PSUM
  softmax division:  reciprocal of row 64, broadcast via K=1 matmul with a
                     ones vector, multiply on VectorE
  proj partial:      lhsT = a^T pair tile [128, 128], rhs = c_proj slice
"""

import os
import sys

import numpy as np

if "/opt/trn_rl_repo" not in sys.path:
    sys.path.insert(0, "/opt/trn_rl_repo")

import ml_dtypes

import concourse.bass as bass
import concourse.mybir as mybir
import concourse.tile as tile
from concourse import bacc
from concourse.bass_utils import run_bass_kernel_spmd

F32 = mybir.dt.float32
F32R = mybir.dt.float32r
BF16 = mybir.dt.bfloat16

B, S, D, H, HD = 2, 2048, 1024, 16, 64
N_CORES = 8
HPC = 4                       # heads per core
QKR = 2 * HPC * HD            # qk^T rows per core (q:256 + k:256) = 512
VC = HPC * HD                 # v columns per core = 256
NKC = D // 128                # contraction chunks over embedding = 8
NJC = S // 128                # key/seq chunks of 128 = 16
NIC = S // 512                # query chunks of 512 = 4
SCALE = 0.125                 # 1/sqrt(hd)

_CACHE: dict = {}


def _build_masks_bf16() -> np.ndarray:
    """masks[m][p, f] = 1.0 if (f >= p + 128*m) else 0 - the causal mask for a
    scores^T block whose key chunk is the (4*ic + m)-th within query chunk ic."""
    p = np.arange(128)[:, None]
    f = np.arange(512)[None, :]
    out = np.zeros((4, 128, 512), np.float32)
    for m in range(4):
        out[m] = (f >= p + 128 * m).astype(np.float32)
    return out.astype(ml_dtypes.bfloat16)


def _emit(tc: tile.TileContext, ctx, tensors):
    nc = tc.nc
    xT_d, wqk_d, wv_d, bqk_d, bv_d, wp_d, ones_d, masks_d, out_d, kT_d, v_d = tensors

    def r(ap):
        return ap

    persist = ctx.enter_context(tc.tile_pool(name="persist", bufs=1))
    epool = ctx.enter_context(tc.tile_pool(name="epool", bufs=6))
    small = ctx.enter_context(tc.tile_pool(name="small", bufs=3))
    outp = ctx.enter_context(tc.tile_pool(name="outp", bufs=3))
    ps_mm = ctx.enter_context(tc.tile_pool(name="ps_mm", bufs=2, space="PSUM"))
    ps_sc = ctx.enter_context(tc.tile_pool(name="ps_sc", bufs=2, space="PSUM"))
    ps_avA = ctx.enter_context(tc.tile_pool(name="ps_avA", bufs=1, space="PSUM"))
    ps_avB = ctx.enter_context(tc.tile_pool(name="ps_avB", bufs=1, space="PSUM"))

    # ---- persistent SBUF tiles -------------------------------------------
    wqk_sb = persist.tile([128, NKC, QKR], F32R, tag="wqk", name="wqk_sb")
    wv_sb = persist.tile([128, NKC, VC], F32R, tag="wv", name="wv_sb")
    bqk_sb = persist.tile([128, QKR // 128], F32, tag="bqk", name="bqk_sb")
    bv_sb = persist.tile([1, VC], F32R, tag="bv", name="bv_sb")
    wp_sb = persist.tile([128, 2, D], F32R, tag="wp", name="wp_sb")
    masks_sb = persist.tile([128, 4, 512], BF16, tag="masks", name="masks_sb")
    ones_sb = persist.tile([128, 128], F32R, tag="ones", name="ones_sb")
    xT_sb = persist.tile([128, NKC, S], F32R, tag="xT", name="xT_sb")
    # qk^T tiles: [0]=q heads(0,1), [1]=q heads(2,3), [2]=k heads(0,1), [3]=k(2,3)
    qkT_sb = [
        persist.tile([128, S], F32R, tag=f"qkT{m}", name=f"qkT{m}") for m in range(4)
    ]
    # v with ones column, bf16: [128p(seq within chunk), jc, head, 65]
    v65_sb = persist.tile([128, NJC, HPC, 65], BF16, tag="v65", name="v65_sb")
    # a^T per head pair: rows 0-63 head 2*pair dims, 64-127 head 2*pair+1
    aT_sb = [
        persist.tile([128, S], F32R, tag=f"aT{p}", name=f"aT{p}") for p in range(2)
    ]

    # ---- input DMAs -------------------------------------------------------
    # order matters for the head of the schedule: wqk + the n=0 quarter of
    # xT land first so the first qk^T pass can start ~7us in.
    wqk_r = wqk_d.rearrange("(ko p) m -> p ko m", p=128)
    nc.scalar.dma_start(wqk_sb[:, :, 0:128], wqk_r[:, :, 0:128])
    for n in range(4):
        for k in range(NKC):
            if n == 0:
                eng = (nc.sync, nc.scalar, nc.gpsimd)[k % 3]
            else:
                eng = nc.sync if k % 2 == 0 else nc.scalar
            eng.dma_start(
                xT_sb[:, k, n * 512:(n + 1) * 512],
                xT_d[k * 128:(k + 1) * 128, n * 512:(n + 1) * 512],
            )
        if n == 0:
            nc.sync.dma_start(wqk_sb[:, :, 256:384], wqk_r[:, :, 256:384])
            for m in (1, 3):
                nc.scalar.dma_start(wqk_sb[:, :, m * 128:(m + 1) * 128],
                                    wqk_r[:, :, m * 128:(m + 1) * 128])
            nc.sync.dma_start(bqk_sb[:], bqk_d.rearrange("(m p) -> p m", p=128))
            nc.sync.dma_start(wv_sb[:], wv_d.rearrange("(ko p) m -> p ko m", p=128))
            nc.sync.dma_start(ones_sb[:], ones_d[:])
            nc.sync.dma_start(bv_sb[:], bv_d[None, :])
            nc.sync.dma_start(masks_sb[:], masks_d.rearrange("m p f -> p m f"))
        if n == 1:
            nc.sync.dma_start(wp_sb[:], wp_d.rearrange("(ko p) n -> p ko n", p=128))
    nc.vector.memset(v65_sb[:, :, :, 64:65], 1.0)
    # warm the ScalarE exp table during the DMA phase (~2.7us table load)
    warm = small.tile([1, 4], F32, tag="warm", name="warm_sb")
    nc.vector.memset(warm[:], 0.0)
    nc.scalar.activation(warm[:], warm[:], mybir.ActivationFunctionType.Exp)

    def emit_qkT(m, n):
        ps = ps_mm.tile([128, 512], F32, tag="mm", name=f"qk_ps_{m}_{n}")
        for k in range(NKC):
            nc.tensor.matmul(
                ps[:],
                wqk_sb[:, k, m * 128:(m + 1) * 128],
                xT_sb[:, k, n * 512:(n + 1) * 512],
                start=(k == 0),
                stop=(k == NKC - 1),
            )
        nc.vector.tensor_scalar(
            qkT_sb[m][:, n * 512:(n + 1) * 512],
            ps[:],
            bqk_sb[:, m:m + 1],
            None,
            mybir.AluOpType.add,
        )

    def emit_v(sc):
        ps = ps_mm.tile([128, 512], F32, tag="mm", name=f"v_ps_{sc}")
        psv = ps[:, 0:VC]
        for k in range(NKC):
            nc.tensor.matmul(
                psv,
                xT_sb[:, k, sc * 128:(sc + 1) * 128],
                wv_sb[:, k, :],
                start=(k == 0),
                stop=False,
            )
        # bias via K=1 rank-1 update: ones[128] x bv[256]
        nc.tensor.matmul(
            psv, ones_sb[0:1, 0:128], bv_sb[0:1, :], start=False, stop=True
        )
        vout = outp.tile([128, VC], F32, tag="vout", name=f"vout_{sc}")
        nc.vector.tensor_copy(vout[:], psv)
        nc.gpsimd.dma_start(v_d[sc * 128:(sc + 1) * 128, :], vout[:])
        nc.vector.tensor_copy(
            v65_sb[:, sc, :, 0:64], psv.rearrange("p (h d) -> p h d", h=HPC)
        )

    def emit_attention(pair, ic):
        qT = qkT_sb[pair]
        kT = qkT_sb[2 + pair]
        aT = aT_sb[pair]
        njc = 4 * ic + 4
        avA = ps_avA.tile([65, 512], F32, tag="avA", name=f"avA_{pair}_{ic}")
        avB = ps_avB.tile([65, 512], F32, tag="avB", name=f"avB_{pair}_{ic}")
        for jc in range(njc):
            # one 2-bank scores tile per key chunk: [A, B]; bufs=2 pipelines
            # the next chunk's matmuls against this chunk's exp
            sc2 = ps_sc.tile([128, 2, 512], F32, tag="sc", name=f"sc_{pair}_{ic}_{jc}")
            nc.tensor.matmul(
                sc2[:, 0, :],
                kT[0:64, jc * 128:(jc + 1) * 128],
                qT[0:64, ic * 512:(ic + 1) * 512],
            )
            nc.tensor.matmul(
                sc2[:, 1, :],
                kT[64:128, jc * 128:(jc + 1) * 128],
                qT[64:128, ic * 512:(ic + 1) * 512],
            )
            e2 = epool.tile([128, 2, 512], BF16, tag="e", name=f"e_{pair}_{ic}_{jc}")
            nc.scalar.activation(
                e2[:], sc2[:], mybir.ActivationFunctionType.Exp, scale=SCALE
            )
            mloc = jc - 4 * ic
            if mloc >= 0:  # diagonal block: zero out future positions
                nc.vector.tensor_tensor(
                    e2[:, :, :], e2[:, :, :],
                    masks_sb[:, mloc:mloc + 1, :].to_broadcast([128, 2, 512]),
                    mybir.AluOpType.mult,
                )
            nc.tensor.matmul(
                avA[:],
                v65_sb[:, jc, 2 * pair, :],
                e2[:, 0, :],
                start=(jc == 0),
                stop=(jc == njc - 1),
            )
            nc.tensor.matmul(
                avB[:],
                v65_sb[:, jc, 2 * pair + 1, :],
                e2[:, 1, :],
                start=(jc == 0),
                stop=(jc == njc - 1),
            )
        # epilogue: divide by the softmax denominator (row 64)
        ics = slice(ic * 512, (ic + 1) * 512)
        for hb, av in ((0, avA), (1, avB)):
            rr = small.tile([65, 512], F32R, tag="recip", name=f"r_{pair}_{ic}_{hb}")
            with nc.allow_low_precision(reason="fp32r rhs of broadcast matmul"):
                nc.vector.reciprocal(rr[64:65, :], av[64:65, :])
            bc_ps = ps_mm.tile([128, 512], F32, tag="mm", name=f"bc_ps_{pair}_{ic}_{hb}")
            nc.tensor.matmul(bc_ps[:], ones_sb[64:65, 0:128], rr[64:65, :])
            bc = small.tile([128, 512], F32, tag="bcast", name=f"bc_{pair}_{ic}_{hb}")
            nc.vector.tensor_copy(bc[:], bc_ps[:])
            if hb == 0:
                nc.vector.tensor_tensor(
                    aT[0:64, ics], av[0:64, :], bc[0:64, :], mybir.AluOpType.mult
                )
            else:
                tmp = small.tile([64, 512], F32R, tag="tmpB", name=f"tmpB_{pair}_{ic}")
                nc.vector.tensor_tensor(
                    tmp[0:64, :], av[0:64, :], bc[0:64, :], mybir.AluOpType.mult
                )
                # shift to partitions 64-127 of the pair tile (DMA crossbar)
                nc.scalar.dma_start(aT[64:128, ics], tmp[0:64, :])

    def emit_proj(i):
        osb = outp.tile([128, D], F32, tag="osb", name=f"osb_{i}")
        for n in range(2):
            ps = ps_mm.tile([128, 512], F32, tag="mm", name=f"o_ps_{i}_{n}")
            for kc in range(2):
                nc.tensor.matmul(
                    ps[:],
                    aT_sb[kc][:, i * 128:(i + 1) * 128],
                    wp_sb[:, kc, n * 512:(n + 1) * 512],
                    start=(kc == 0),
                    stop=(kc == 1),
                )
            cp = nc.any if i >= S // 128 - 4 else nc.vector
            cp.tensor_copy(osb[:, n * 512:(n + 1) * 512], ps[:])
        nc.gpsimd.dma_start(out_d[i * 128:(i + 1) * 128, :], osb[:])

    # ---- main schedule: ic-outer; the next chunk's qk^T/v are emitted
    # before this chunk's proj so the PE always has projection work queued
    # while the attention epilogues drain.
    for m in (0, 2, 1, 3):
        emit_qkT(m, 0)
    for sc in range(4):
        emit_v(sc)
    for ic in range(NIC):
        for pair in range(2):
            emit_attention(pair, ic)
        if ic + 1 < NIC:
            for m in (0, 2, 1, 3):
                emit_qkT(m, ic + 1)
            for sc in range(4 * ic + 4, 4 * ic + 8):
                emit_v(sc)
        if ic + 1 == NIC - 1:
            # present: k^T rows are qkT tiles 2 and 3 - emit as soon as the
            # last qk^T passes exist so the DMA drains during attention
            nc.gpsimd.dma_start(kT_d[0:128, :], qkT_sb[2][:])
            nc.gpsimd.dma_start(kT_d[128:256, :], qkT_sb[3][:])
        for i in range(4 * ic, 4 * ic + 4):
            emit_proj(i)


def build_nc():
    if "nc" in _CACHE:
        return _CACHE["nc"]
    from contextlib import ExitStack

    nc = bacc.Bacc(None, target_bir_lowering=False)
    xT_d = nc.dram_tensor("xT", [D, S], F32R, kind="ExternalInput")
    wqk_d = nc.dram_tensor("wqk", [D, QKR], F32R, kind="ExternalInput")
    wv_d = nc.dram_tensor("wv", [D, VC], F32R, kind="ExternalInput")
    bqk_d = nc.dram_tensor("bqk", [QKR], F32, kind="ExternalInput")
    bv_d = nc.dram_tensor("bv", [VC], F32R, kind="ExternalInput")
    wp_d = nc.dram_tensor("wp", [VC, D], F32R, kind="ExternalInput")
    ones_d = nc.dram_tensor("ones", [128, 128], F32R, kind="ExternalInput")
    masks_d = nc.inline_tensor(_build_masks_bf16(), name="masks")
    out_d = nc.dram_tensor("out_p", [S, D], F32, kind="ExternalOutput")
    kT_d = nc.dram_tensor("kT_out", [VC, S], F32R, kind="ExternalOutput")
    v_d = nc.dram_tensor("v_out", [S, VC], F32, kind="ExternalOutput")

    tensors = (xT_d[:], wqk_d[:], wv_d[:], bqk_d[:], bv_d[:], wp_d[:],
               ones_d[:], masks_d[:], out_d[:], kT_d[:], v_d[:])
    with tile.TileContext(nc) as tc:
        with ExitStack() as ctx:
            _emit(tc, ctx, tensors)
    nc.compile()
    _CACHE["nc"] = nc
    return nc


def make_in_maps(x, c_attn_w, c_attn_b, c_proj_w):
    x = np.asarray(x, np.float32)
    c_attn_w = np.asarray(c_attn_w, np.float32)
    c_attn_b = np.asarray(c_attn_b, np.float32)
    c_proj_w = np.asarray(c_proj_w, np.float32)
    in_maps = []
    for c in range(N_CORES):
        b, hg = divmod(c, 4)
        qs = slice(VC * hg, VC * (hg + 1))
        ks = slice(D + VC * hg, D + VC * (hg + 1))
        vs = slice(2 * D + VC * hg, 2 * D + VC * (hg + 1))
        in_maps.append({
            "xT": np.ascontiguousarray(x[b].T),
            "wqk": np.ascontiguousarray(
                np.concatenate([c_attn_w[:, qs], c_attn_w[:, ks]], axis=1)),
            "wv": np.ascontiguousarray(c_attn_w[:, vs]),
            "bqk": np.ascontiguousarray(
                np.concatenate([c_attn_b[qs], c_attn_b[ks]])),
            "bv": np.ascontiguousarray(c_attn_b[vs]),
            "wp": np.ascontiguousarray(c_proj_w[VC * hg:VC * (hg + 1), :]),
            "ones": np.ones((128, 128), np.float32),
        })
    return in_maps


def gather(results, c_proj_b):
    c_proj_b = np.asarray(c_proj_b, np.float32)
    a = np.zeros((B, S, D), np.float32)
    present = np.empty((2, B, H, S, HD), np.float32)
    for c in range(N_CORES):
        b, hg = divmod(c, 4)
        rs = results[c]
        a[b] += rs["out_p"]
        hsl = slice(HPC * hg, HPC * (hg + 1))
        present[0, b, hsl] = (
            rs["kT_out"].reshape(HPC, HD, S).transpose(0, 2, 1))
        present[1, b, hsl] = (
            rs["v_out"].reshape(S, HPC, HD).transpose(1, 0, 2))
    a += c_proj_b
    return a, present


def run(in_maps, trace=False, **kw):
    nc = build_nc()
    if not trace:
        # this container has no NTFF hook (antenv.axon_hooks absent); a
        # BASS_TRACE env var would crash the axon trace path, so pin it off
        os.environ.setdefault("BASS_NEVER_TRACE", "1")
    return run_bass_kernel_spmd(nc, in_maps, list(range(N_CORES)), trace=trace, **kw)


def kernel(x, c_attn_w, c_attn_b, c_proj_w, c_proj_b):
    in_maps = make_in_maps(x, c_attn_w, c_attn_b, c_proj_w)
    res = run(in_maps)
    return gather(res.results, c_proj_b)


# revision 20
# speedup vs baseline: 1.0933x; 1.0708x over previous
"""Trainium2 Bass kernel for a GPT-style causal multi-head attention block.

Reference computation (per problem nn_Attention_45286135169078):
    qkv = x @ c_attn_w + c_attn_b              # [B,S,3D]
    q,k,v -> heads [B,H,S,hd], causal softmax(q k^T / sqrt(hd)) @ v
    a = merge_heads @ c_proj_w + c_proj_b      # [B,S,D]
    present = stack(k_heads, v_heads)          # [2,B,H,S,hd]
    returns (a, present)

Sharding across 8 NeuronCores: (batch b, head-group hg) with b in {0,1} and
hg in {0..3}; each core handles 4 heads of one batch (tensor-parallel over
heads x data-parallel over batch).  c_attn columns / c_proj rows are split by
head on the host; the c_proj partial outputs are summed on the host (the
"all-reduce after c_proj" of the hint, done at gather time).

Per-core device kernel (all matmuls on fp32 data run in float32r mode, the
exp->AV path runs in bf16):
  qk^T = (x Wqk)^T  [512, 2048]  - q rows 0..255, k rows 256..511, with the
                                   two heads of a "pair" stacked in one
                                   128-partition tile
  v    = x Wv       [2048, 256]  - natural layout, plus a ones column per
                                   head -> AV matmul also produces softmax
                                   denominators (M=65)
  scores^T blocks [128 j, 512 i] - lhsT = k^T slice (K=64), two heads packed
                                   into the PE array via row tile_position
  e = exp(scores/8)  on ScalarE, PSUM->SBUF, bf16, 2 blocks per instruction
  causal masking     0/1 bf16 mask multiply on diagonal blocks only
  AV: lhsT = [v | 1] [128, 65], rhs = e block -> accumulate [65, 512] in PSUM
  softmax division:  reciprocal of row 64, broadcast via K=1 matmul with a
                     ones vector, multiply on VectorE
  proj partial:      lhsT = a^T pair tile [128, 128], rhs = c_proj slice
"""

import os
import sys

import numpy as np

if "/opt/trn_rl_repo" not in sys.path:
    sys.path.insert(0, "/opt/trn_rl_repo")

import ml_dtypes

import concourse.bass as bass
import concourse.mybir as mybir
import concourse.tile as tile
from concourse import bacc
from concourse.bass_utils import run_bass_kernel_spmd

F32 = mybir.dt.float32
F32R = mybir.dt.float32r
BF16 = mybir.dt.bfloat16

B, S, D, H, HD = 2, 2048, 1024, 16, 64
N_CORES = 8
HPC = 4                       # heads per core
QKR = 2 * HPC * HD            # qk^T rows per core (q:256 + k:256) = 512
VC = HPC * HD                 # v columns per core = 256
NKC = D // 128                # contraction chunks over embedding = 8
NJC = S // 128                # key/seq chunks of 128 = 16
NIC = S // 512                # query chunks of 512 = 4
SCALE = 0.125                 # 1/sqrt(hd)

_CACHE: dict = {}


def _build_masks_bf16() -> np.ndarray:
    """masks[m][p, f] = 1.0 if (f >= p + 128*m) else 0 - the causal mask for a
    scores^T block whose key chunk is the (4*ic + m)-th within query chunk ic."""
    p = np.arange(128)[:, None]
    f = np.arange(512)[None, :]
    out = np.zeros((4, 128, 512), np.float32)
    for m in range(4):
        out[m] = (f >= p + 128 * m).astype(np.float32)
    return out.astype(ml_dtypes.bfloat16)


def _emit(tc: tile.TileContext, ctx, tensors):
    nc = tc.nc
    xT_d, wqk_d, wv_d, bqk_d, bv_d, wp_d, ones_d, masks_d, out_d, kT_d, v_d = tensors

    def r(ap):
        return ap

    persist = ctx.enter_context(tc.tile_pool(name="persist", bufs=1))
    epool = ctx.enter_context(tc.tile_pool(name="epool", bufs=6))
    small = ctx.enter_context(tc.tile_pool(name="small", bufs=3))
    outp = ctx.enter_context(tc.tile_pool(name="outp", bufs=3))
    ps_mm = ctx.enter_context(tc.tile_pool(name="ps_mm", bufs=2, space="PSUM"))
    ps_sc = ctx.enter_context(tc.tile_pool(name="ps_sc", bufs=2, space="PSUM"))
    ps_avA = ctx.enter_context(tc.tile_pool(name="ps_avA", bufs=1, space="PSUM"))
    ps_avB = ctx.enter_context(tc.tile_pool(name="ps_avB", bufs=1, space="PSUM"))

    # ---- persistent SBUF tiles -------------------------------------------
    wqk_sb = persist.tile([128, NKC, QKR], F32R, tag="wqk", name="wqk_sb")
    wv_sb = persist.tile([128, NKC, VC], F32R, tag="wv", name="wv_sb")
    bqk_sb = persist.tile([128, QKR // 128], F32, tag="bqk", name="bqk_sb")
    bv_sb = persist.tile([1, VC], F32R, tag="bv", name="bv_sb")
    wp_sb = persist.tile([128, 2, D], F32R, tag="wp", name="wp_sb")
    masks_sb = persist.tile([128, 4, 512], BF16, tag="masks", name="masks_sb")
    ones_sb = persist.tile([128, 128], F32R, tag="ones", name="ones_sb")
    xT_sb = persist.tile([128, NKC, S], F32R, tag="xT", name="xT_sb")
    # qk^T tiles: [0]=q heads(0,1), [1]=q heads(2,3), [2]=k heads(0,1), [3]=k(2,3)
    qkT_sb = [
        persist.tile([128, S], F32R, tag=f"qkT{m}", name=f"qkT{m}") for m in range(4)
    ]
    # v with ones column, bf16: [128p(seq within chunk), jc, head, 65]
    v65_sb = persist.tile([128, NJC, HPC, 65], BF16, tag="v65", name="v65_sb")
    # a^T per head pair: rows 0-63 head 2*pair dims, 64-127 head 2*pair+1
    aT_sb = [
        persist.tile([128, S], F32R, tag=f"aT{p}", name=f"aT{p}") for p in range(2)
    ]

    # ---- input DMAs -------------------------------------------------------
    # order matters for the head of the schedule: wqk + the n=0 quarter of
    # xT land first so the first qk^T pass can start ~7us in.
    wqk_r = wqk_d.rearrange("(ko p) m -> p ko m", p=128)
    nc.scalar.dma_start(wqk_sb[:, :, 0:128], wqk_r[:, :, 0:128])
    for n in range(4):
        for k in range(NKC):
            if n == 0:
                eng = (nc.sync, nc.scalar, nc.gpsimd)[k % 3]
            else:
                eng = nc.sync if k % 2 == 0 else nc.scalar
            eng.dma_start(
                xT_sb[:, k, n * 512:(n + 1) * 512],
                xT_d[k * 128:(k + 1) * 128, n * 512:(n + 1) * 512],
            )
        if n == 0:
            nc.sync.dma_start(wqk_sb[:, :, 256:384], wqk_r[:, :, 256:384])
            for m in (1, 3):
                nc.scalar.dma_start(wqk_sb[:, :, m * 128:(m + 1) * 128],
                                    wqk_r[:, :, m * 128:(m + 1) * 128])
            nc.sync.dma_start(bqk_sb[:], bqk_d.rearrange("(m p) -> p m", p=128))
            nc.sync.dma_start(wv_sb[:], wv_d.rearrange("(ko p) m -> p ko m", p=128))
            nc.sync.dma_start(ones_sb[:], ones_d[:])
            nc.sync.dma_start(bv_sb[:], bv_d[None, :])
            nc.sync.dma_start(masks_sb[:], masks_d.rearrange("m p f -> p m f"))
        if n == 1:
            nc.sync.dma_start(wp_sb[:], wp_d.rearrange("(ko p) n -> p ko n", p=128))
    nc.vector.memset(v65_sb[:, :, :, 64:65], 1.0)
    # warm the ScalarE exp table during the DMA phase (~2.7us table load)
    warm = small.tile([1, 4], F32, tag="warm", name="warm_sb")
    nc.vector.memset(warm[:], 0.0)
    nc.scalar.activation(warm[:], warm[:], mybir.ActivationFunctionType.Exp)

    def emit_qkT(m, n):
        ps = ps_mm.tile([128, 512], F32, tag="mm", name=f"qk_ps_{m}_{n}")
        for k in range(NKC):
            nc.tensor.matmul(
                ps[:],
                wqk_sb[:, k, m * 128:(m + 1) * 128],
                xT_sb[:, k, n * 512:(n + 1) * 512],
                start=(k == 0),
                stop=(k == NKC - 1),
            )
        nc.vector.tensor_scalar(
            qkT_sb[m][:, n * 512:(n + 1) * 512],
            ps[:],
            bqk_sb[:, m:m + 1],
            None,
            mybir.AluOpType.add,
        )

    def emit_v(sc):
        ps = ps_mm.tile([128, 512], F32, tag="mm", name=f"v_ps_{sc}")
        psv = ps[:, 0:VC]
        for k in range(NKC):
            nc.tensor.matmul(
                psv,
                xT_sb[:, k, sc * 128:(sc + 1) * 128],
                wv_sb[:, k, :],
                start=(k == 0),
                stop=False,
            )
        # bias via K=1 rank-1 update: ones[128] x bv[256]
        nc.tensor.matmul(
            psv, ones_sb[0:1, 0:128], bv_sb[0:1, :], start=False, stop=True
        )
        vout = outp.tile([128, VC], F32, tag="vout", name=f"vout_{sc}")
        nc.vector.tensor_copy(vout[:], psv)
        nc.gpsimd.dma_start(v_d[sc * 128:(sc + 1) * 128, :], vout[:])
        nc.vector.tensor_copy(
            v65_sb[:, sc, :, 0:64], psv.rearrange("p (h d) -> p h d", h=HPC)
        )

    def emit_attention(pair, ic):
        qT = qkT_sb[pair]
        kT = qkT_sb[2 + pair]
        aT = aT_sb[pair]
        njc = 4 * ic + 4
        avA = ps_avA.tile([65, 512], F32, tag="avA", name=f"avA_{pair}_{ic}")
        avB = ps_avB.tile([65, 512], F32, tag="avB", name=f"avB_{pair}_{ic}")
        for jc in range(njc):
            # one 2-bank scores tile per key chunk: [A, B]; bufs=2 pipelines
            # the next chunk's matmuls against this chunk's exp
            sc2 = ps_sc.tile([128, 2, 512], F32, tag="sc", name=f"sc_{pair}_{ic}_{jc}")
            nc.tensor.matmul(
                sc2[:, 0, :],
                kT[0:64, jc * 128:(jc + 1) * 128],
                qT[0:64, ic * 512:(ic + 1) * 512],
            )
            nc.tensor.matmul(
                sc2[:, 1, :],
                kT[64:128, jc * 128:(jc + 1) * 128],
                qT[64:128, ic * 512:(ic + 1) * 512],
            )
            e2 = epool.tile([128, 2, 512], BF16, tag="e", name=f"e_{pair}_{ic}_{jc}")
            nc.scalar.activation(
                e2[:], sc2[:], mybir.ActivationFunctionType.Exp, scale=SCALE
            )
            mloc = jc - 4 * ic
            if mloc >= 0:  # diagonal block: zero out future positions
                nc.vector.tensor_tensor(
                    e2[:, :, :], e2[:, :, :],
                    masks_sb[:, mloc:mloc + 1, :].to_broadcast([128, 2, 512]),
                    mybir.AluOpType.mult,
                )
            nc.tensor.matmul(
                avA[:],
                v65_sb[:, jc, 2 * pair, :],
                e2[:, 0, :],
                start=(jc == 0),
                stop=(jc == njc - 1),
            )
            nc.tensor.matmul(
                avB[:],
                v65_sb[:, jc, 2 * pair + 1, :],
                e2[:, 1, :],
                start=(jc == 0),
                stop=(jc == njc - 1),
            )
        # epilogue: divide by the softmax denominator (row 64)
        ics = slice(ic * 512, (ic + 1) * 512)
        for hb, av in ((0, avA), (1, avB)):
            rr = small.tile([65, 512], F32R, tag="recip", name=f"r_{pair}_{ic}_{hb}")
            with nc.allow_low_precision(reason="fp32r rhs of broadcast matmul"):
                nc.vector.reciprocal(rr[64:65, :], av[64:65, :])
            bc_ps = ps_sc.tile([128, 512], F32, tag="sc", name=f"bc_ps_{pair}_{ic}_{hb}")
            nc.tensor.matmul(bc_ps[:], ones_sb[64:65, 0:128], rr[64:65, :])
            bc = small.tile([128, 512], F32, tag="bcast", name=f"bc_{pair}_{ic}_{hb}")
            nc.vector.tensor_copy(bc[:], bc_ps[:])
            if hb == 0:
                nc.vector.tensor_tensor(
                    aT[0:64, ics], av[0:64, :], bc[0:64, :], mybir.AluOpType.mult
                )
            else:
                tmp = small.tile([64, 512], F32R, tag="tmpB", name=f"tmpB_{pair}_{ic}")
                nc.vector.tensor_tensor(
                    tmp[0:64, :], av[0:64, :], bc[0:64, :], mybir.AluOpType.mult
                )
                # shift to partitions 64-127 of the pair tile (DMA crossbar)
                nc.scalar.dma_start(aT[64:128, ics], tmp[0:64, :])

    def emit_proj(i):
        osb = outp.tile([128, D], F32, tag="osb", name=f"osb_{i}")
        for n in range(2):
            ps = ps_mm.tile([128, 512], F32, tag="mm", name=f"o_ps_{i}_{n}")
            for kc in range(2):
                nc.tensor.matmul(
                    ps[:],
                    aT_sb[kc][:, i * 128:(i + 1) * 128],
                    wp_sb[:, kc, n * 512:(n + 1) * 512],
                    start=(kc == 0),
                    stop=(kc == 1),
                )
            cp = nc.any if i >= S // 128 - 4 else nc.vector
            cp.tensor_copy(osb[:, n * 512:(n + 1) * 512], ps[:])
        nc.gpsimd.dma_start(out_d[i * 128:(i + 1) * 128, :], osb[:])

    # ---- main schedule: ic-outer; the next chunk's qk^T/v are emitted
    # before this chunk's proj so the PE always has projection work queued
    # while the attention epilogues drain.
    for m in (0, 2, 1, 3):
        emit_qkT(m, 0)
    for sc in range(4):
        emit_v(sc)
    for ic in range(NIC):
        for pair in range(2):
            emit_attention(pair, ic)
        if ic + 1 < NIC:
            for m in (0, 2, 1, 3):
                emit_qkT(m, ic + 1)
            for sc in range(4 * ic + 4, 4 * ic + 8):
                emit_v(sc)
        if ic + 1 == NIC - 1:
            # present: k^T rows are qkT tiles 2 and 3 - emit as soon as the
            # last qk^T passes exist so the DMA drains during attention
            nc.gpsimd.dma_start(kT_d[0:128, :], qkT_sb[2][:])
            nc.gpsimd.dma_start(kT_d[128:256, :], qkT_sb[3][:])
        for i in range(4 * ic, 4 * ic + 4):
            emit_proj(i)


def build_nc():
    if "nc" in _CACHE:
        return _CACHE["nc"]
    from contextlib import ExitStack

    nc = bacc.Bacc(None, target_bir_lowering=False)
    xT_d = nc.dram_tensor("xT", [D, S], F32R, kind="ExternalInput")
    wqk_d = nc.dram_tensor("wqk", [D, QKR], F32R, kind="ExternalInput")
    wv_d = nc.dram_tensor("wv", [D, VC], F32R, kind="ExternalInput")
    bqk_d = nc.dram_tensor("bqk", [QKR], F32, kind="ExternalInput")
    bv_d = nc.dram_tensor("bv", [VC], F32R, kind="ExternalInput")
    wp_d = nc.dram_tensor("wp", [VC, D], F32R, kind="ExternalInput")
    ones_d = nc.dram_tensor("ones", [128, 128], F32R, kind="ExternalInput")
    masks_d = nc.inline_tensor(_build_masks_bf16(), name="masks")
    out_d = nc.dram_tensor("out_p", [S, D], F32, kind="ExternalOutput")
    kT_d = nc.dram_tensor("kT_out", [VC, S], F32R, kind="ExternalOutput")
    v_d = nc.dram_tensor("v_out", [S, VC], F32, kind="ExternalOutput")

    tensors = (xT_d[:], wqk_d[:], wv_d[:], bqk_d[:], bv_d[:], wp_d[:],
               ones_d[:], masks_d[:], out_d[:], kT_d[:], v_d[:])
    with tile.TileContext(nc) as tc:
        with ExitStack() as ctx:
            _emit(tc, ctx, tensors)
    nc.compile()
    _CACHE["nc"] = nc
    return nc


def make_in_maps(x, c_attn_w, c_attn_b, c_proj_w):
    x = np.asarray(x, np.float32)
    c_attn_w = np.asarray(c_attn_w, np.float32)
    c_attn_b = np.asarray(c_attn_b, np.float32)
    c_proj_w = np.asarray(c_proj_w, np.float32)
    in_maps = []
    for c in range(N_CORES):
        b, hg = divmod(c, 4)
        qs = slice(VC * hg, VC * (hg + 1))
        ks = slice(D + VC * hg, D + VC * (hg + 1))
        vs = slice(2 * D + VC * hg, 2 * D + VC * (hg + 1))
        in_maps.append({
            "xT": np.ascontiguousarray(x[b].T),
            "wqk": np.ascontiguousarray(
                np.concatenate([c_attn_w[:, qs], c_attn_w[:, ks]], axis=1)),
            "wv": np.ascontiguousarray(c_attn_w[:, vs]),
            "bqk": np.ascontiguousarray(
                np.concatenate([c_attn_b[qs], c_attn_b[ks]])),
            "bv": np.ascontiguousarray(c_attn_b[vs]),
            "wp": np.ascontiguousarray(c_proj_w[VC * hg:VC * (hg + 1), :]),
            "ones": np.ones((128, 128), np.float32),
        })
    return in_maps


def gather(results, c_proj_b):
    c_proj_b = np.asarray(c_proj_b, np.float32)
    a = np.zeros((B, S, D), np.float32)
    present = np.empty((2, B, H, S, HD), np.float32)
    for c in range(N_CORES):
        b, hg = divmod(c, 4)
        rs = results[c]
        a[b] += rs["out_p"]
        hsl = slice(HPC * hg, HPC * (hg + 1))
        present[0, b, hsl] = (
            rs["kT_out"].reshape(HPC, HD, S).transpose(0, 2, 1))
        present[1, b, hsl] = (
            rs["v_out"].reshape(S, HPC, HD).transpose(1, 0, 2))
    a += c_proj_b
    return a, present


def run(in_maps, trace=False, **kw):
    nc = build_nc()
    if not trace:
        # this container has no NTFF hook (antenv.axon_hooks absent); a
        # BASS_TRACE env var would crash the axon trace path, so pin it off
        os.environ.setdefault("BASS_NEVER_TRACE", "1")
    return run_bass_kernel_spmd(nc, in_maps, list(range(N_CORES)), trace=trace, **kw)


def kernel(x, c_attn_w, c_attn_b, c_proj_w, c_proj_b):
    in_maps = make_in_maps(x, c_attn_w, c_attn_b, c_proj_w)
    res = run(in_maps)
    return gather(res.results, c_proj_b)
